# revision 24
# baseline (speedup 1.0000x reference)
"""Bezier surface fitter as a sharded matmul on 8 TRN2 NeuronCores.

out[b,c,h,w] = sum_{p,q} basis[h*w, p, q] * K[b, c, p, q]

Fast path (used when the provided basis is verifiably rank-1 separable,
basis[(i,j),p,q] == F[i,p]*G[j,q], which the Bernstein tensor basis is by
construction): host precomputes A[bc,i,q] = sum_p F[i,p] K[bc,p,q]; the
device only expands out[bc, i*W+j] = sum_q A[bc,i,q] G[j,q] and streams the
128 MB fp32 result out — the kernel is output-DMA-bound at the ~450 GB/s
per-core SBUF->HBM ceiling (~37 us for 16.78 MB/core).

General fallback (non-separable basis): plain tiled matmul
OUT[bc, n] = KF[bc, k] @ BF[n, k]^T with k=256 on SBUF partitions, n (=h*w)
sharded across the 8 cores per the sharding hint.
"""

import os

import numpy as np

import concourse.bass as bass
import concourse.mybir as mybir
from concourse import bacc
from concourse.bass_utils import run_bass_kernel_spmd
from concourse.tile import TileContext

N_CORES = 8
B, C, H, W, M1, N1 = 8, 16, 512, 512, 16, 16
BC = B * C            # 128
KDIM = M1 * N1        # 256
HW = H * W            # 262144
SHARD = HW // N_CORES  # 32768

NT = 2048             # output columns per outer tile (psum tile = 4 banks)
MM_N = 512            # moving free dim per matmul (one psum bank of f32)
KCHUNKS = KDIM // 128  # 2

MM_DTYPE = mybir.dt.float32  # switchable: float32 | float32r | bfloat16

LAST_RESULT = None  # BassKernelResults of the most recent run (for test harness)


def _build_nc(mm_dtype=None, nt=None, b_bufs=4, o_bufs=4, p_bufs=2, repeats=1, _alt=False):
    mm_dtype = MM_DTYPE if mm_dtype is None else mm_dtype
    io_dtype = mm_dtype if mm_dtype == mybir.dt.float32r else mybir.dt.float32
    global NT
    NT_save = NT
    if nt is not None:
        NT = nt
    nc = bacc.Bacc()
    kt = nc.declare_dram_parameter("kt", [KDIM, BC], io_dtype, isOutput=False)
    if _alt:  # test-harness only: alternate weights across repeats
        kt2 = nc.declare_dram_parameter("kt2", [KDIM, BC], io_dtype, isOutput=False)
    bt = nc.declare_dram_parameter("bt", [KDIM, SHARD], io_dtype, isOutput=False)
    out = nc.declare_dram_parameter("out", [BC, SHARD], mybir.dt.float32, isOutput=True)

    n_tiles = SHARD // NT
    with TileContext(nc) as tc:
        with (
            tc.tile_pool(name="kpool", bufs=1) as kpool,
            tc.tile_pool(name="bpool", bufs=b_bufs) as bpool,
            tc.tile_pool(name="opool", bufs=o_bufs) as opool,
            tc.tile_pool(name="ppool", bufs=p_bufs, space="PSUM") as ppool,
        ):
            ktile = kpool.tile([128, KCHUNKS * BC], io_dtype)
            nc.sync.dma_start(
                out=ktile[:, :].rearrange("p (c m) -> p c m", c=KCHUNKS),
                in_=kt[:, :].rearrange("(c p) m -> p c m", p=128),
            )
            if _alt:
                ktile2 = kpool.tile([128, KCHUNKS * BC], io_dtype)
                nc.sync.dma_start(
                    out=ktile2[:, :].rearrange("p (c m) -> p c m", c=KCHUNKS),
                    in_=kt2[:, :].rearrange("(c p) m -> p c m", p=128),
                )
            for _rep in range(repeats):
                kt_use = ktile2 if (_alt and _rep % 2) else ktile
                for t in range(n_tiles):
                    btile = bpool.tile([128, KCHUNKS * NT], io_dtype)
                    nc.sync.dma_start(
                        out=btile[:, :].rearrange("p (c n) -> p c n", c=KCHUNKS),
                        in_=bt[:, :].rearrange("(c p) n -> p c n", p=128)[
                            :, :, t * NT : (t + 1) * NT
                        ],
                    )
                    ptile = ppool.tile([128, NT], mybir.dt.float32)
                    for j in range(NT // MM_N):
                        for c in range(KCHUNKS):
                            nc.tensor.matmul(
                                ptile[:, j * MM_N : (j + 1) * MM_N],
                                lhsT=kt_use[:, c * BC : (c + 1) * BC].bitcast(mm_dtype),
                                rhs=btile[
                                    :, c * NT + j * MM_N : c * NT + (j + 1) * MM_N
                                ].bitcast(mm_dtype),
                                start=(c == 0),
                                stop=(c == KCHUNKS - 1),
                            )
                    otile = opool.tile([128, NT], mybir.dt.float32)
                    nc.vector.tensor_copy(otile[:, :], ptile[:, :])
                    nc.sync.dma_start(
                        out=out[:, t * NT : (t + 1) * NT], in_=otile[:, :]
                    )
    NT = NT_save
    nc.finalize()
    return nc


ILOC = H // N_CORES  # 64 rows of the h-grid per core on the fast path

# --- fast2 path geometry ---
# src tensor [128, 2W + ILOC*BC/4] f16.  Matmul operands must start at
# partition 0/32/64, so i-rows are packed two-pairs-per-128-col-block:
# block bb rows 0-63 hold [Ah;Al] of i=4bb,4bb+1 and rows 64-127 of
# i=4bb+2,4bb+3.  Each matmul is K=64 at base 0 or 64 whose rhs selects one
# pair member by zero-padding: cols 0..W-1 = [Gh;Gh;0;0], W..2W-1 =
# [0;0;Gh;Gh] (replicated on both partition halves).  Every DMA spans all
# 128 partitions (full port bandwidth); the zero rows stream for free.
F2_GROUPS = (1, 1, 2, 4, 8, 8, 8, 8, 8, 8, 8)  # i-rows per out tile (ramp), Σ=64
F2_AST0 = 2 * W  # ast payload starts after the two rhs selector blocks
F2_AW = 1152  # srca: [selA | selB | blk0] — loaded first, single contiguous DMA
F2_BW = ILOC * BC // 4 - BC  # srcb: blk1..blk15, overlapped with early groups


def _build_nc_fast2(o_bufs=4, p_bufs=2, repeats=1, _alt=False, groups=None,
                    in_loop=False, vsplit=4, dma_alt=False):
    """Two-term f16 hi/lo expansion: out_i = [Ah_i;Al_i]^T @ [Gh;Gh].

    i-rows are pair-stacked across partition halves (see the geometry note
    above F2_GROUPS) so both input tensors load with one full-width,
    fully-contiguous, 128-partition DMA each.  Output tiles ramp
    1,1,2,4,8.. i-rows: the first out-DMA issues ~2us in, while steady
    state streams 2 MB DMAs (1 MB DMAs measured ~15% slower).
    """
    f16 = mybir.dt.float16
    nc = bacc.Bacc()
    srcs = {}
    for nm, wd in (("srca", F2_AW), ("srcb", F2_BW)):
        srcs[nm] = nc.declare_dram_parameter(nm, [128, wd], f16, isOutput=False)
        if _alt and in_loop:
            srcs[nm + "2"] = nc.declare_dram_parameter(
                nm + "2", [128, wd], f16, isOutput=False
            )
    out = nc.declare_dram_parameter("out", [BC, SHARD], mybir.dt.float32, isOutput=True)

    with TileContext(nc) as tc:
        with (
            tc.tile_pool(name="cpool", bufs=2) as cpool,
            tc.tile_pool(name="opool", bufs=o_bufs) as opool,
            tc.tile_pool(name="ppool", bufs=p_bufs, space="PSUM") as ppool,
        ):
            groups_use = F2_GROUPS if groups is None else groups
            atile = btile = None
            for _rep in range(repeats):
                sfx = "2" if (_alt and _rep % 2) else ""
                if in_loop or _rep == 0:
                    # full-width DMAs: the DRAM side is fully contiguous, so
                    # each lands as a few large descriptors at line rate
                    atile = cpool.tile([128, F2_AW], f16)
                    nc.sync.dma_start(out=atile[:, :], in_=srcs["srca" + sfx][:, :])
                    btile = cpool.tile([128, F2_BW], f16)
                    nc.sync.dma_start(out=btile[:, :], in_=srcs["srcb" + sfx][:, :])
                i0 = 0
                for gi, og in enumerate(groups_use):
                    otile = opool.tile([128, og * W], mybir.dt.float32)
                    for s2 in range(0, og, 4):
                        pg = min(og - s2, 4)
                        ptile = ppool.tile([128, 2048], mybir.dt.float32)
                        for u in range(pg):
                            i = i0 + s2 + u
                            w = i % 4
                            half = 64 * (w // 2)
                            bb = i // 4
                            lhsT = (
                                atile[half : half + 64, F2_AST0 : F2_AST0 + BC]
                                if bb == 0
                                else btile[half : half + 64, (bb - 1) * BC : bb * BC]
                            )
                            nc.tensor.matmul(
                                ptile[:, u * W : (u + 1) * W],
                                lhsT=lhsT,
                                rhs=atile[half : half + 64, (w % 2) * W : (w % 2 + 1) * W],
                                start=True,
                                stop=True,
                            )
                        # PSUM->SBUF copy split across DVE and ACT; ~50/50 is
                        # balanced (fp32 PSUM-source: DVE 1x @0.96 GHz vs
                        # ScalarE @1.2 GHz)
                        cols = pg * W
                        vc = (cols * vsplit // 8) // 64 * 64
                        nc.vector.tensor_copy(
                            otile[:, s2 * W : s2 * W + vc], ptile[:, :vc]
                        )
                        nc.scalar.copy(
                            otile[:, s2 * W + vc : (s2 + pg) * W], ptile[:, vc:cols]
                        )
                    dma_eng = nc.scalar if (dma_alt and gi % 2) else nc.sync
                    dma_eng.dma_start(
                        out=out[:, i0 * W : (i0 + og) * W], in_=otile[:, :]
                    )
                    i0 += og
    nc.finalize()
    return nc


def _build_nc_fast(o_bufs=5, p_bufs=8, repeats=1, OG=4, PG=4, _alt=False):
    """Fast path: basis is separable (basis[(i,j),p,q] = F[i,p] * G[j,q]).

    Host precomputes A[bc,i,q] = sum_p F[i,p] * K[bc,p,q]; the device only
    expands out[bc, i*W+j] = sum_q A[bc,i,q] * G[j,q] — then streams results
    out.  A and G are shipped as float16 hi/lo split pairs, STACKED along the
    contraction dim: lhsT = [Ah; Al; Ah; Al] (K=64), rhs = [Gh; Gh; Gl; Gl],
    so one 512-cycle f16 matmul per output block computes all four product
    terms (full fp32-equivalent accuracy, ~2^-22), vs fp32's 4 cycles/col.
    Per-core inputs: ast = stacked A^T slice [64, ILOC*128] f16,
                     bst = stacked G^T [64, W] f16.
    """
    f16 = mybir.dt.float16
    nc = bacc.Bacc()
    ast = nc.declare_dram_parameter("ast", [64, ILOC * BC], f16, isOutput=False)
    bst = nc.declare_dram_parameter("bst", [64, W], f16, isOutput=False)
    if _alt:  # test-harness only: alternate rhs across repeats so no repeat
        bst2 = nc.declare_dram_parameter("bst2", [64, W], f16, isOutput=False)
    out = nc.declare_dram_parameter("out", [BC, SHARD], mybir.dt.float32, isOutput=True)

    with TileContext(nc) as tc:
        with (
            tc.tile_pool(name="cpool", bufs=1) as cpool,
            tc.tile_pool(name="opool", bufs=o_bufs) as opool,
            tc.tile_pool(name="ppool", bufs=p_bufs // PG, space="PSUM") as ppool,
        ):
            bsttile = cpool.tile([64, W], f16)
            nc.sync.dma_start(out=bsttile[:, :], in_=bst[:, :])
            if _alt:
                bsttile2 = cpool.tile([64, W], f16)
                nc.sync.dma_start(out=bsttile2[:, :], in_=bst2[:, :])
            asttile = cpool.tile([64, ILOC * BC], f16)
            CH = ILOC // 16
            for c in range(16):
                nc.sync.dma_start(
                    out=asttile[:, c * CH * BC : (c + 1) * CH * BC],
                    in_=ast[:, c * CH * BC : (c + 1) * CH * BC],
                )
            for _rep in range(repeats):
                bt_use = bsttile2 if (_alt and _rep % 2) else bsttile
                for g in range(ILOC // OG):
                    otile = opool.tile([128, OG * W], mybir.dt.float32)
                    for s2 in range(OG // PG):
                        ptile = ppool.tile([128, PG * W], mybir.dt.float32)
                        for u in range(PG):
                            il = g * OG + s2 * PG + u
                            nc.tensor.matmul(
                                ptile[:, u * W : (u + 1) * W],
                                lhsT=asttile[:, il * BC : (il + 1) * BC],
                                rhs=bt_use[:, :],
                                start=True,
                                stop=True,
                            )
                        # split the PSUM->SBUF copy across VectorE and ScalarE
                        # (parallel: the halves live in different PSUM banks);
                        # the serial DVE copy chain otherwise binds the kernel
                        half = PG * W // 2
                        nc.vector.tensor_copy(
                            otile[:, s2 * PG * W : s2 * PG * W + half],
                            ptile[:, :half],
                        )
                        nc.scalar.copy(
                            otile[:, s2 * PG * W + half : (s2 + 1) * PG * W],
                            ptile[:, half:],
                        )
                    nc.sync.dma_start(
                        out=out[:, g * OG * W : (g + 1) * OG * W], in_=otile[:, :]
                    )
    nc.finalize()
    return nc


def _try_separate(basis4):
    """If basis[(i,j),p,q] == F[i,p] * G[j,q] (to fp32 accuracy), return
    (F, G) as float64 arrays; else None.  Exact-by-construction check: the
    factorization is verified elementwise against the provided data."""
    S = basis4.sum(axis=(1, 3), dtype=np.float64)  # [H, M1] = F * sum(G)
    T = basis4.sum(axis=(0, 2), dtype=np.float64)  # [W, N1] = G * sum(F)
    tot = float(S.sum())
    if not np.isfinite(tot) or abs(tot) < 1e-30:
        return None
    F = S
    G = T / tot
    scale = float(np.max(np.abs(basis4)))
    if scale == 0.0 or not np.isfinite(scale):
        return None
    # chunked elementwise verification of the reconstruction.  A truly
    # separable f32 tensor reconstructs to ~3e-8 * scale (f32 rounding);
    # 1e-6 leaves margin while rejecting anything meaningfully non-rank-1.
    for i0 in range(0, H, 64):
        rec = np.einsum(
            "ip,jq->ijpq", F[i0 : i0 + 64], G, optimize=True
        ).astype(np.float32)
        err = np.max(np.abs(rec - basis4[i0 : i0 + 64]))
        if not (err <= 1e-6 * scale):
            return None
    return F, G


def kernel(K: np.ndarray, basis: np.ndarray) -> np.ndarray:
    global LAST_RESULT
    K = np.ascontiguousarray(np.asarray(K, dtype=np.float32))
    basis = np.asarray(basis, dtype=np.float32)

    force = os.environ.get("BASS_KERNEL_FORCE", "")  # "", "fast", "general"
    fact = None
    if force != "general":
        fact = _try_separate(basis.reshape(H, W, M1, N1))

    trace = os.environ.get("BASS_KERNEL_TRACE", "0") == "1"
    core_ids = list(range(N_CORES))

    if fact is not None:
        try:
            return _run_fast(K, fact, core_ids, trace)
        except Exception:
            pass  # graceful degradation: fall through to the general path
    return _run_general(K, basis, core_ids, trace)


def _f2_in_maps(K, fact):
    """Per-core {"srca": [128, F2_AW], "srcb": [128, F2_BW]} f16 inputs."""
    F, G = fact
    # rebalance so both factors are O(1): the f16 hi/lo split loses
    # precision badly when one factor carries a ~512x scale
    c = float(np.max(np.abs(F)))
    F = F / c
    G = G * c
    # A[bc, i, q] = sum_p F[i,p] * K[bc,p,q]
    A = np.einsum(
        "ip,bpq->biq", F, K.reshape(BC, M1, N1).astype(np.float64), optimize=True
    ).astype(np.float32)
    GhT = G.astype(np.float32).astype(np.float16).T  # [16, W]
    z32 = np.zeros((32, W), np.float16)
    selA = np.concatenate([GhT, GhT, z32], axis=0)  # [64, W]
    selB = np.concatenate([z32, GhT, GhT], axis=0)  # [64, W]
    bst = np.tile(np.concatenate([selA, selB], axis=1), (2, 1))  # [128, 2W]
    Ah = A.astype(np.float16)
    Al = (A - Ah.astype(np.float32)).astype(np.float16)
    # ast partition row = 64*half + 32*member + 16*s + r for
    # i = c*64 + bb*4 + 2*half + member; col = bb*BC + m
    H1 = np.stack(
        [
            Ah.reshape(BC, N_CORES, 16, 2, 2, 16),
            Al.reshape(BC, N_CORES, 16, 2, 2, 16),
        ],
        axis=0,
    )  # [s, m, c, bb, half, member, r]
    ast = np.ascontiguousarray(
        H1.transpose(2, 4, 5, 0, 6, 3, 1).reshape(N_CORES, 128, ILOC * BC // 4)
    )
    return [
        {
            "srca": np.ascontiguousarray(
                np.concatenate([bst, ast[i, :, :BC]], axis=1)
            ),
            "srcb": np.ascontiguousarray(ast[i, :, BC:]),
        }
        for i in range(N_CORES)
    ]


def _run_fast(K, fact, core_ids, trace):
    global LAST_RESULT
    in_maps = _f2_in_maps(K, fact)
    nc = _build_nc_fast2()
    LAST_RESULT = run_bass_kernel_spmd(nc, in_maps, core_ids=core_ids, trace=trace)
    res = LAST_RESULT.results
    out = np.concatenate([res[i]["out"] for i in range(N_CORES)], axis=1)  # [128, HW]
    return out.reshape(1, B, C, H, W)


def _run_general(K, basis, core_ids, trace):
    global LAST_RESULT
    kt_full = np.ascontiguousarray(K.reshape(BC, KDIM).T)  # [256, 128]
    bflat = basis.reshape(HW, KDIM)
    in_maps = []
    for i in range(N_CORES):
        bt_i = np.ascontiguousarray(
            bflat[i * SHARD : (i + 1) * SHARD].T
        )  # [256, SHARD]
        in_maps.append({"kt": kt_full, "bt": bt_i})
    nc = _build_nc(nt=1024, b_bufs=4, o_bufs=4, p_bufs=2)
    LAST_RESULT = run_bass_kernel_spmd(nc, in_maps, core_ids=core_ids, trace=trace)
    res = LAST_RESULT.results
    out = np.concatenate([res[i]["out"] for i in range(N_CORES)], axis=1)  # [128, HW]
    return out.reshape(1, B, C, H, W)



# revision 25
# speedup vs baseline: 1.0096x; 1.0096x over previous
"""Bezier surface fitter as a sharded matmul on 8 TRN2 NeuronCores.

out[b,c,h,w] = sum_{p,q} basis[h*w, p, q] * K[b, c, p, q]

Fast path (used when the provided basis is verifiably rank-1 separable,
basis[(i,j),p,q] == F[i,p]*G[j,q], which the Bernstein tensor basis is by
construction): host precomputes A[bc,i,q] = sum_p F[i,p] K[bc,p,q]; the
device only expands out[bc, i*W+j] = sum_q A[bc,i,q] G[j,q] and streams the
128 MB fp32 result out — the kernel is output-DMA-bound at the ~450 GB/s
per-core SBUF->HBM ceiling (~37 us for 16.78 MB/core).

General fallback (non-separable basis): plain tiled matmul
OUT[bc, n] = KF[bc, k] @ BF[n, k]^T with k=256 on SBUF partitions, n (=h*w)
sharded across the 8 cores per the sharding hint.
"""

import os

import numpy as np

import concourse.bass as bass
import concourse.mybir as mybir
from concourse import bacc
from concourse.bass_utils import run_bass_kernel_spmd
from concourse.tile import TileContext

N_CORES = 8
B, C, H, W, M1, N1 = 8, 16, 512, 512, 16, 16
BC = B * C            # 128
KDIM = M1 * N1        # 256
HW = H * W            # 262144
SHARD = HW // N_CORES  # 32768

NT = 2048             # output columns per outer tile (psum tile = 4 banks)
MM_N = 512            # moving free dim per matmul (one psum bank of f32)
KCHUNKS = KDIM // 128  # 2

MM_DTYPE = mybir.dt.float32  # switchable: float32 | float32r | bfloat16

LAST_RESULT = None  # BassKernelResults of the most recent run (for test harness)


def _build_nc(mm_dtype=None, nt=None, b_bufs=4, o_bufs=4, p_bufs=2, repeats=1, _alt=False):
    mm_dtype = MM_DTYPE if mm_dtype is None else mm_dtype
    io_dtype = mm_dtype if mm_dtype == mybir.dt.float32r else mybir.dt.float32
    global NT
    NT_save = NT
    if nt is not None:
        NT = nt
    nc = bacc.Bacc()
    kt = nc.declare_dram_parameter("kt", [KDIM, BC], io_dtype, isOutput=False)
    if _alt:  # test-harness only: alternate weights across repeats
        kt2 = nc.declare_dram_parameter("kt2", [KDIM, BC], io_dtype, isOutput=False)
    bt = nc.declare_dram_parameter("bt", [KDIM, SHARD], io_dtype, isOutput=False)
    out = nc.declare_dram_parameter("out", [BC, SHARD], mybir.dt.float32, isOutput=True)

    n_tiles = SHARD // NT
    with TileContext(nc) as tc:
        with (
            tc.tile_pool(name="kpool", bufs=1) as kpool,
            tc.tile_pool(name="bpool", bufs=b_bufs) as bpool,
            tc.tile_pool(name="opool", bufs=o_bufs) as opool,
            tc.tile_pool(name="ppool", bufs=p_bufs, space="PSUM") as ppool,
        ):
            ktile = kpool.tile([128, KCHUNKS * BC], io_dtype)
            nc.sync.dma_start(
                out=ktile[:, :].rearrange("p (c m) -> p c m", c=KCHUNKS),
                in_=kt[:, :].rearrange("(c p) m -> p c m", p=128),
            )
            if _alt:
                ktile2 = kpool.tile([128, KCHUNKS * BC], io_dtype)
                nc.sync.dma_start(
                    out=ktile2[:, :].rearrange("p (c m) -> p c m", c=KCHUNKS),
                    in_=kt2[:, :].rearrange("(c p) m -> p c m", p=128),
                )
            for _rep in range(repeats):
                kt_use = ktile2 if (_alt and _rep % 2) else ktile
                for t in range(n_tiles):
                    btile = bpool.tile([128, KCHUNKS * NT], io_dtype)
                    nc.sync.dma_start(
                        out=btile[:, :].rearrange("p (c n) -> p c n", c=KCHUNKS),
                        in_=bt[:, :].rearrange("(c p) n -> p c n", p=128)[
                            :, :, t * NT : (t + 1) * NT
                        ],
                    )
                    ptile = ppool.tile([128, NT], mybir.dt.float32)
                    for j in range(NT // MM_N):
                        for c in range(KCHUNKS):
                            nc.tensor.matmul(
                                ptile[:, j * MM_N : (j + 1) * MM_N],
                                lhsT=kt_use[:, c * BC : (c + 1) * BC].bitcast(mm_dtype),
                                rhs=btile[
                                    :, c * NT + j * MM_N : c * NT + (j + 1) * MM_N
                                ].bitcast(mm_dtype),
                                start=(c == 0),
                                stop=(c == KCHUNKS - 1),
                            )
                    otile = opool.tile([128, NT], mybir.dt.float32)
                    nc.vector.tensor_copy(otile[:, :], ptile[:, :])
                    nc.sync.dma_start(
                        out=out[:, t * NT : (t + 1) * NT], in_=otile[:, :]
                    )
    NT = NT_save
    nc.finalize()
    return nc


ILOC = H // N_CORES  # 64 rows of the h-grid per core on the fast path

# --- fast2 path geometry ---
# src tensor [128, 2W + ILOC*BC/4] f16.  Matmul operands must start at
# partition 0/32/64, so i-rows are packed two-pairs-per-128-col-block:
# block bb rows 0-63 hold [Ah;Al] of i=4bb,4bb+1 and rows 64-127 of
# i=4bb+2,4bb+3.  Each matmul is K=64 at base 0 or 64 whose rhs selects one
# pair member by zero-padding: cols 0..W-1 = [Gh;Gh;0;0], W..2W-1 =
# [0;0;Gh;Gh] (replicated on both partition halves).  Every DMA spans all
# 128 partitions (full port bandwidth); the zero rows stream for free.
F2_GROUPS = (1, 1, 2, 4, 8, 8, 8, 8, 8, 8, 8)  # i-rows per out tile (ramp), Σ=64
F2_AST0 = 2 * W  # ast payload starts after the two rhs selector blocks
F2_AW = 1152  # srca: [selA | selB | blk0] — loaded first, single contiguous DMA
F2_BW = ILOC * BC // 4 - BC  # srcb: blk1..blk15, overlapped with early groups


def _build_nc_fast2(o_bufs=4, p_bufs=2, repeats=1, _alt=False, groups=None,
                    in_loop=False, vsplit=4, dma_alt=False):
    """Two-term f16 hi/lo expansion: out_i = [Ah_i;Al_i]^T @ [Gh;Gh].

    i-rows are pair-stacked across partition halves (see the geometry note
    above F2_GROUPS) so both input tensors load with one full-width,
    fully-contiguous, 128-partition DMA each.  Output tiles ramp
    1,1,2,4,8.. i-rows: the first out-DMA issues ~2us in, while steady
    state streams 2 MB DMAs (1 MB DMAs measured ~15% slower).
    """
    f16 = mybir.dt.float16
    nc = bacc.Bacc()
    srcs = {}
    for nm, wd in (("srca", F2_AW), ("srcb", F2_BW)):
        srcs[nm] = nc.declare_dram_parameter(nm, [128, wd], f16, isOutput=False)
        if _alt and in_loop:
            srcs[nm + "2"] = nc.declare_dram_parameter(
                nm + "2", [128, wd], f16, isOutput=False
            )
    out = nc.declare_dram_parameter("out", [BC, SHARD], mybir.dt.float32, isOutput=True)

    with TileContext(nc) as tc:
        with (
            tc.tile_pool(name="cpool", bufs=2) as cpool,
            tc.tile_pool(name="opool", bufs=o_bufs) as opool,
            tc.tile_pool(name="ppool", bufs=p_bufs, space="PSUM") as ppool,
        ):
            groups_use = F2_GROUPS if groups is None else groups
            # prime ScalarE's activation table (LoadActFuncSet, ~1.3us) with a
            # dummy copy at t~0 so the first real PSUM->SBUF copy doesn't pay
            # it on the critical head path; overlaps the input DMAs.
            scratch = cpool.tile([128, 64], mybir.dt.float32)
            nc.vector.memset(scratch[:, :32], 0.0)
            nc.scalar.copy(scratch[:, 32:], scratch[:, :32])
            atile = btile = None
            for _rep in range(repeats):
                sfx = "2" if (_alt and _rep % 2) else ""
                if in_loop or _rep == 0:
                    # full-width DMAs: the DRAM side is fully contiguous, so
                    # each lands as a few large descriptors at line rate
                    atile = cpool.tile([128, F2_AW], f16)
                    nc.sync.dma_start(out=atile[:, :], in_=srcs["srca" + sfx][:, :])
                    btile = cpool.tile([128, F2_BW], f16)
                    nc.sync.dma_start(out=btile[:, :], in_=srcs["srcb" + sfx][:, :])
                i0 = 0
                for gi, og in enumerate(groups_use):
                    otile = opool.tile([128, og * W], mybir.dt.float32)
                    for s2 in range(0, og, 4):
                        pg = min(og - s2, 4)
                        ptile = ppool.tile([128, 2048], mybir.dt.float32)
                        for u in range(pg):
                            i = i0 + s2 + u
                            w = i % 4
                            half = 64 * (w // 2)
                            bb = i // 4
                            lhsT = (
                                atile[half : half + 64, F2_AST0 : F2_AST0 + BC]
                                if bb == 0
                                else btile[half : half + 64, (bb - 1) * BC : bb * BC]
                            )
                            nc.tensor.matmul(
                                ptile[:, u * W : (u + 1) * W],
                                lhsT=lhsT,
                                rhs=atile[half : half + 64, (w % 2) * W : (w % 2 + 1) * W],
                                start=True,
                                stop=True,
                            )
                        # PSUM->SBUF copy split across DVE and ACT; ~50/50 is
                        # balanced (fp32 PSUM-source: DVE 1x @0.96 GHz vs
                        # ScalarE @1.2 GHz)
                        cols = pg * W
                        vc = (cols * vsplit // 8) // 64 * 64
                        nc.vector.tensor_copy(
                            otile[:, s2 * W : s2 * W + vc], ptile[:, :vc]
                        )
                        nc.scalar.copy(
                            otile[:, s2 * W + vc : (s2 + pg) * W], ptile[:, vc:cols]
                        )
                    dma_eng = nc.scalar if (dma_alt and gi % 2) else nc.sync
                    dma_eng.dma_start(
                        out=out[:, i0 * W : (i0 + og) * W], in_=otile[:, :]
                    )
                    i0 += og
    nc.finalize()
    return nc


def _build_nc_fast(o_bufs=5, p_bufs=8, repeats=1, OG=4, PG=4, _alt=False):
    """Fast path: basis is separable (basis[(i,j),p,q] = F[i,p] * G[j,q]).

    Host precomputes A[bc,i,q] = sum_p F[i,p] * K[bc,p,q]; the device only
    expands out[bc, i*W+j] = sum_q A[bc,i,q] * G[j,q] — then streams results
    out.  A and G are shipped as float16 hi/lo split pairs, STACKED along the
    contraction dim: lhsT = [Ah; Al; Ah; Al] (K=64), rhs = [Gh; Gh; Gl; Gl],
    so one 512-cycle f16 matmul per output block computes all four product
    terms (full fp32-equivalent accuracy, ~2^-22), vs fp32's 4 cycles/col.
    Per-core inputs: ast = stacked A^T slice [64, ILOC*128] f16,
                     bst = stacked G^T [64, W] f16.
    """
    f16 = mybir.dt.float16
    nc = bacc.Bacc()
    ast = nc.declare_dram_parameter("ast", [64, ILOC * BC], f16, isOutput=False)
    bst = nc.declare_dram_parameter("bst", [64, W], f16, isOutput=False)
    if _alt:  # test-harness only: alternate rhs across repeats so no repeat
        bst2 = nc.declare_dram_parameter("bst2", [64, W], f16, isOutput=False)
    out = nc.declare_dram_parameter("out", [BC, SHARD], mybir.dt.float32, isOutput=True)

    with TileContext(nc) as tc:
        with (
            tc.tile_pool(name="cpool", bufs=1) as cpool,
            tc.tile_pool(name="opool", bufs=o_bufs) as opool,
            tc.tile_pool(name="ppool", bufs=p_bufs // PG, space="PSUM") as ppool,
        ):
            bsttile = cpool.tile([64, W], f16)
            nc.sync.dma_start(out=bsttile[:, :], in_=bst[:, :])
            if _alt:
                bsttile2 = cpool.tile([64, W], f16)
                nc.sync.dma_start(out=bsttile2[:, :], in_=bst2[:, :])
            asttile = cpool.tile([64, ILOC * BC], f16)
            CH = ILOC // 16
            for c in range(16):
                nc.sync.dma_start(
                    out=asttile[:, c * CH * BC : (c + 1) * CH * BC],
                    in_=ast[:, c * CH * BC : (c + 1) * CH * BC],
                )
            for _rep in range(repeats):
                bt_use = bsttile2 if (_alt and _rep % 2) else bsttile
                for g in range(ILOC // OG):
                    otile = opool.tile([128, OG * W], mybir.dt.float32)
                    for s2 in range(OG // PG):
                        ptile = ppool.tile([128, PG * W], mybir.dt.float32)
                        for u in range(PG):
                            il = g * OG + s2 * PG + u
                            nc.tensor.matmul(
                                ptile[:, u * W : (u + 1) * W],
                                lhsT=asttile[:, il * BC : (il + 1) * BC],
                                rhs=bt_use[:, :],
                                start=True,
                                stop=True,
                            )
                        # split the PSUM->SBUF copy across VectorE and ScalarE
                        # (parallel: the halves live in different PSUM banks);
                        # the serial DVE copy chain otherwise binds the kernel
                        half = PG * W // 2
                        nc.vector.tensor_copy(
                            otile[:, s2 * PG * W : s2 * PG * W + half],
                            ptile[:, :half],
                        )
                        nc.scalar.copy(
                            otile[:, s2 * PG * W + half : (s2 + 1) * PG * W],
                            ptile[:, half:],
                        )
                    nc.sync.dma_start(
                        out=out[:, g * OG * W : (g + 1) * OG * W], in_=otile[:, :]
                    )
    nc.finalize()
    return nc


def _try_separate(basis4):
    """If basis[(i,j),p,q] == F[i,p] * G[j,q] (to fp32 accuracy), return
    (F, G) as float64 arrays; else None.  Exact-by-construction check: the
    factorization is verified elementwise against the provided data."""
    S = basis4.sum(axis=(1, 3), dtype=np.float64)  # [H, M1] = F * sum(G)
    T = basis4.sum(axis=(0, 2), dtype=np.float64)  # [W, N1] = G * sum(F)
    tot = float(S.sum())
    if not np.isfinite(tot) or abs(tot) < 1e-30:
        return None
    F = S
    G = T / tot
    scale = float(np.max(np.abs(basis4)))
    if scale == 0.0 or not np.isfinite(scale):
        return None
    # chunked elementwise verification of the reconstruction.  A truly
    # separable f32 tensor reconstructs to ~3e-8 * scale (f32 rounding);
    # 1e-6 leaves margin while rejecting anything meaningfully non-rank-1.
    for i0 in range(0, H, 64):
        rec = np.einsum(
            "ip,jq->ijpq", F[i0 : i0 + 64], G, optimize=True
        ).astype(np.float32)
        err = np.max(np.abs(rec - basis4[i0 : i0 + 64]))
        if not (err <= 1e-6 * scale):
            return None
    return F, G


def kernel(K: np.ndarray, basis: np.ndarray) -> np.ndarray:
    global LAST_RESULT
    K = np.ascontiguousarray(np.asarray(K, dtype=np.float32))
    basis = np.asarray(basis, dtype=np.float32)

    force = os.environ.get("BASS_KERNEL_FORCE", "")  # "", "fast", "general"
    fact = None
    if force != "general":
        fact = _try_separate(basis.reshape(H, W, M1, N1))

    trace = os.environ.get("BASS_KERNEL_TRACE", "0") == "1"
    core_ids = list(range(N_CORES))

    if fact is not None:
        try:
            return _run_fast(K, fact, core_ids, trace)
        except Exception:
            pass  # graceful degradation: fall through to the general path
    return _run_general(K, basis, core_ids, trace)


def _f2_in_maps(K, fact):
    """Per-core {"srca": [128, F2_AW], "srcb": [128, F2_BW]} f16 inputs."""
    F, G = fact
    # rebalance so both factors are O(1): the f16 hi/lo split loses
    # precision badly when one factor carries a ~512x scale
    c = float(np.max(np.abs(F)))
    F = F / c
    G = G * c
    # A[bc, i, q] = sum_p F[i,p] * K[bc,p,q]
    A = np.einsum(
        "ip,bpq->biq", F, K.reshape(BC, M1, N1).astype(np.float64), optimize=True
    ).astype(np.float32)
    GhT = G.astype(np.float32).astype(np.float16).T  # [16, W]
    z32 = np.zeros((32, W), np.float16)
    selA = np.concatenate([GhT, GhT, z32], axis=0)  # [64, W]
    selB = np.concatenate([z32, GhT, GhT], axis=0)  # [64, W]
    bst = np.tile(np.concatenate([selA, selB], axis=1), (2, 1))  # [128, 2W]
    Ah = A.astype(np.float16)
    Al = (A - Ah.astype(np.float32)).astype(np.float16)
    # ast partition row = 64*half + 32*member + 16*s + r for
    # i = c*64 + bb*4 + 2*half + member; col = bb*BC + m
    H1 = np.stack(
        [
            Ah.reshape(BC, N_CORES, 16, 2, 2, 16),
            Al.reshape(BC, N_CORES, 16, 2, 2, 16),
        ],
        axis=0,
    )  # [s, m, c, bb, half, member, r]
    ast = np.ascontiguousarray(
        H1.transpose(2, 4, 5, 0, 6, 3, 1).reshape(N_CORES, 128, ILOC * BC // 4)
    )
    return [
        {
            "srca": np.ascontiguousarray(
                np.concatenate([bst, ast[i, :, :BC]], axis=1)
            ),
            "srcb": np.ascontiguousarray(ast[i, :, BC:]),
        }
        for i in range(N_CORES)
    ]


def _run_fast(K, fact, core_ids, trace):
    global LAST_RESULT
    in_maps = _f2_in_maps(K, fact)
    nc = _build_nc_fast2()
    LAST_RESULT = run_bass_kernel_spmd(nc, in_maps, core_ids=core_ids, trace=trace)
    res = LAST_RESULT.results
    out = np.concatenate([res[i]["out"] for i in range(N_CORES)], axis=1)  # [128, HW]
    return out.reshape(1, B, C, H, W)


def _run_general(K, basis, core_ids, trace):
    global LAST_RESULT
    kt_full = np.ascontiguousarray(K.reshape(BC, KDIM).T)  # [256, 128]
    bflat = basis.reshape(HW, KDIM)
    in_maps = []
    for i in range(N_CORES):
        bt_i = np.ascontiguousarray(
            bflat[i * SHARD : (i + 1) * SHARD].T
        )  # [256, SHARD]
        in_maps.append({"kt": kt_full, "bt": bt_i})
    nc = _build_nc(nt=1024, b_bufs=4, o_bufs=4, p_bufs=2)
    LAST_RESULT = run_bass_kernel_spmd(nc, in_maps, core_ids=core_ids, trace=trace)
    res = LAST_RESULT.results
    out = np.concatenate([res[i]["out"] for i in range(N_CORES)], axis=1)  # [128, HW]
    return out.reshape(1, B, C, H, W)

